# revision 10
# baseline (speedup 1.0000x reference)
import numpy as np
import ml_dtypes
from contextlib import ExitStack

import concourse.mybir as mybir
import concourse.bass as bass
import concourse.tile as tile
from concourse.bass_utils import run_bass_kernel_spmd

# nn_Predictor (moe_routing): L=6 streams, B=16384, D=512, NC=3992, 4 experts,
# hard one-hot gating. Host computes the gate (fp64) and routes: tokens are
# permuted so each core gets ceil(C_e/8) tokens per expert; each token runs
# only its own expert. Expert stage (W1/W2/dec1) runs in fp8e4m3 with
# DoubleRow pairing (weights pre-scaled x64, un-scaled in the activations);
# the decoder dec2 runs in bf16 (error budget), accumulating fp32 in PSUM.
# Host pre-transposes activations to feature-major; dec2 bias added on host.
L, B, D, NCLS, NE = 6, 16384, 512, 3992, 4
NCORES = 8
F32 = mybir.dt.float32
BF16 = mybir.dt.bfloat16
BF = ml_dtypes.bfloat16
F8 = mybir.dt.float8e4
NP8 = ml_dtypes.float8_e4m3
W1_SCALE = 64.0
DR = mybir.MatmulPerfMode.DoubleRow

# (xT row-chunk offset, number of 128-row K chunks) per expert
EXP_K = [(0, 12), (12, 12), (0, 24), (0, 24)]
W1_OFF = [0, 12 * 512, 24 * 512, 48 * 512]   # col offsets into w1img
W1_TOT = 72 * 512


def _split_even(n, maxw=512):
    out = []
    while n > 0:
        t = min(maxw, n)
        out.append(t)
        n -= t
    return out


def _build(ns):
    """ns: per-core token count for each expert (same on all cores)."""
    TOKP = sum(ns)
    NT = TOKP // 128   # leftover (<128) tokens are computed on the host

    nc = bass.Bass("TRN2")
    xT = nc.dram_tensor("xT", [24 * 128, TOKP], F8, kind="ExternalInput")
    w1img = nc.dram_tensor("w1img", [128, W1_TOT], F8, kind="ExternalInput")
    w2img = nc.dram_tensor("w2img", [128, NE * 16 * 128], F8, kind="ExternalInput")
    dw1img = nc.dram_tensor("dw1img", [128, 16 * 128], F8, kind="ExternalInput")
    dw2img = nc.dram_tensor("dw2img", [128, 4 * NCLS], BF16, kind="ExternalInput")
    b1img = nc.dram_tensor("b1img", [128, 16], F32, kind="ExternalInput")
    b2img = nc.dram_tensor("b2img", [128, 16], F32, kind="ExternalInput")
    db1img = nc.dram_tensor("db1img", [128, 4], F32, kind="ExternalInput")
    out = nc.dram_tensor("out", [TOKP, NCLS], BF16, kind="ExternalOutput")

    # subgroups: (expert, token offset, width, first-of-expert)
    subgroups = []
    t0 = 0
    for e in range(NE):
        if ns[e] == 0:
            continue
        for i, T in enumerate(_split_even(ns[e])):
            subgroups.append((e, t0, T, i == 0))
            t0 += T

    with tile.TileContext(nc) as tc, ExitStack() as ctx:
        singles = ctx.enter_context(tc.tile_pool(name="singles", bufs=1))
        xP = ctx.enter_context(tc.tile_pool(name="xP", bufs=3))
        w1P = ctx.enter_context(tc.tile_pool(name="w1P", bufs=3))
        hP = ctx.enter_context(tc.tile_pool(name="hP", bufs=3))
        selP = ctx.enter_context(tc.tile_pool(name="selP", bufs=3))
        outP = ctx.enter_context(tc.tile_pool(name="outP", bufs=4))

        aPs = ctx.enter_context(tc.tile_pool(name="aPs", bufs=4, space="PSUM"))
        oPs = ctx.enter_context(tc.tile_pool(name="oPs", bufs=2, space="PSUM"))

        # tiny bias needed by the very first Relu: load it first on sync q
        b1sb = singles.tile([128, 16], F32)
        nc.sync.dma_start(out=b1sb, in_=b1img[:, :])

        # resident tiles (loads emitted below, in wire-consumption order)
        w2sb = singles.tile([128, NE, 4, 2, 2, 128], F8)
        dw1sb = singles.tile([128, 4, 2, 2, 128], F8)
        b2sb = singles.tile([128, 16], F32)
        db1sb = singles.tile([128, 4], F32)
        dw2sb = singles.tile([128, 4, NCLS], BF16)
        sigAll = singles.tile([128, 4, TOKP], BF16)

        # per-subgroup x / W1 loads, streamed in aligned pieces so the first
        # psum chain starts as data lands. W1/W2/dw1 images are pair-major for
        # DoubleRow: col ((m*nkp + jp)*2 + i)*128 + c = W[(2jp+i)*128+p, m*128+c]
        sub_tiles = {}

        def emit_loads(isub):
            e, t0, T, first = subgroups[isub]
            klo, nk = EXP_K[e]
            nkp = nk // 2
            if first:
                w1t = w1P.tile([128, 4, 12, 2, 128], F8, name="w1t")
                nc.sync.dma_start(
                    out=w1t[:, 0, :nkp, :, :],
                    in_=bass.AP(tensor=w1img, offset=W1_OFF[e],
                                ap=[[W1_TOT, 128], [1, nkp * 256]]),
                )
            else:
                w1t = sub_tiles[isub - 1][0]
            xt = xP.tile([128, 24, 512], F8, name="xt")
            pieces = [2] * 12 if isub == 0 else [6, 6, 6, 6]
            p0 = 0
            for pn in pieces:
                pn = min(pn, nk - p0)
                if pn <= 0:
                    break
                nc.sync.dma_start(
                    out=xt[:, p0: p0 + pn, :T],
                    in_=bass.AP(tensor=xT, offset=(klo + p0) * 128 * TOKP + t0,
                                ap=[[TOKP, 128], [128 * TOKP, pn], [1, T]]),
                )
                p0 += pn
            if first:
                for m in range(1, 4):
                    nc.sync.dma_start(
                        out=w1t[:, m, :nkp, :, :],
                        in_=bass.AP(tensor=w1img,
                                    offset=W1_OFF[e] + m * nkp * 256,
                                    ap=[[W1_TOT, 128], [1, nkp * 256]]),
                    )
                # this expert's W2 block rides along behind its W1
                nc.sync.dma_start(
                    out=w2sb[:, e],
                    in_=bass.AP(tensor=w2img, offset=e * 16 * 128,
                                ap=[[NE * 16 * 128, 128], [1, 16 * 128]]),
                )
            sub_tiles[isub] = (w1t, xt)

        # HAM warmup: keep the PE busy while the first loads are in flight
        # so the clock gate is already at 8/8 when real matmuls start
        warm = singles.tile([128, 128], BF16)
        nc.gpsimd.memset(warm, 0.0)
        wps = aPs.tile([128, 512], F32, name="wps", tag="ps")
        for _ in range(52):
            nc.tensor.matmul(wps[:, :128], warm, warm, start=True, stop=True)

        emit_loads(0)
        nc.sync.dma_start(
            out=dw1sb,
            in_=bass.AP(tensor=dw1img, offset=0, ap=[[16 * 128, 128], [1, 16 * 128]]),
        )
        nc.sync.dma_start(out=b2sb, in_=b2img[:, :])
        nc.sync.dma_start(out=db1sb, in_=db1img[:, :])
        if len(subgroups) > 1:
            emit_loads(1)

        # ---- phase A: per-expert W1 -> relu -> W2 -> +b2 -> dec1 -> sigmoid
        # software-pipelined: subgroup g+1's W1 runs before subgroup g's
        # W2/dec1 so the relu/identity activations are long finished by the
        # time their consumers issue (no PE wait bubbles at stage bounds)
        sub_h = {}

        def stage_w1(isub):
            e, t0, T, first = subgroups[isub]
            klo, nk = EXP_K[e]
            if isub not in sub_tiles:
                emit_loads(isub)
            w1t, xt = sub_tiles[isub]
            nkp = nk // 2
            dr = T >= 256   # DoubleRow only pays off at wide free dims
            h4 = hP.tile([128, 4, 512], F8, name="h4")
            for m in range(4):
                ps = aPs.tile([128, 512], F32, name="hps", tag="ps")
                if dr:
                    for jp in range(nkp):
                        nc.tensor.matmul(
                            ps[:, :T], w1t[:, m, jp, :, :],
                            xt[:, 2 * jp: 2 * jp + 2, :T],
                            start=(jp == 0), stop=(jp == nkp - 1), perf_mode=DR,
                        )
                else:
                    for kj in range(nk):
                        nc.tensor.matmul(
                            ps[:, :T], w1t[:, m, kj // 2, kj % 2, :],
                            xt[:, kj, :T],
                            start=(kj == 0), stop=(kj == nk - 1),
                        )
                nc.scalar.activation(
                    h4[:, m, :T], ps[:, :T], mybir.ActivationFunctionType.Relu,
                    bias=b1sb[:, e * 4 + m: e * 4 + m + 1], scale=1.0 / W1_SCALE,
                )
            sub_h[isub] = h4

        def stage_rest(isub):
            e, t0, T, first = subgroups[isub]
            T_ = T
            dr = T >= 256
            h4 = sub_h.pop(isub)
            sel4 = selP.tile([128, 4, 512], F8, name="sel4")
            for m2 in range(4):
                ps = aPs.tile([128, 512], F32, name="sps", tag="ps")
                if dr:
                    for kp in range(2):
                        nc.tensor.matmul(
                            ps[:, :T], w2sb[:, e, m2, kp, :, :],
                            h4[:, 2 * kp: 2 * kp + 2, :T],
                            start=(kp == 0), stop=(kp == 1), perf_mode=DR,
                        )
                else:
                    for k2 in range(4):
                        nc.tensor.matmul(
                            ps[:, :T], w2sb[:, e, m2, k2 // 2, k2 % 2, :],
                            h4[:, k2, :T],
                            start=(k2 == 0), stop=(k2 == 3),
                        )
                nc.scalar.activation(
                    sel4[:, m2, :T], ps[:, :T],
                    mybir.ActivationFunctionType.Identity,
                    bias=b2sb[:, e * 4 + m2: e * 4 + m2 + 1], scale=1.0 / W1_SCALE,
                )
            for mh in range(4):
                ps = aPs.tile([128, 512], F32, name="dps", tag="ps")
                if dr:
                    for kp in range(2):
                        nc.tensor.matmul(
                            ps[:, :T], dw1sb[:, mh, kp, :, :],
                            sel4[:, 2 * kp: 2 * kp + 2, :T],
                            start=(kp == 0), stop=(kp == 1), perf_mode=DR,
                        )
                else:
                    for kd in range(4):
                        nc.tensor.matmul(
                            ps[:, :T], dw1sb[:, mh, kd // 2, kd % 2, :],
                            sel4[:, kd, :T],
                            start=(kd == 0), stop=(kd == 3),
                        )
                nc.scalar.activation(
                    sigAll[:, mh, t0: t0 + T], ps[:, :T],
                    mybir.ActivationFunctionType.Sigmoid,
                    bias=db1sb[:, mh: mh + 1], scale=1.0 / W1_SCALE,
                )

        # dec2 for token chunk t only needs sigAll[:, :, :t*128+tc]; emit
        # chunk batches as expert subgroups complete so dec2 fills phase A's
        # DMA-paced PE idle. Out DMAs ride the vector queue so they don't
        # reorder the input stream on the sync queue.
        DW2_PIECES = [(0, 1024), (1024, 1024), (2048, 1024), (3072, NCLS - 3072)]
        dw2_emitted = [False] * 4

        def emit_dw2(j):
            if dw2_emitted[j]:
                return
            c0, cw = DW2_PIECES[j]
            nc.sync.dma_start(
                out=dw2sb[:, :, c0: c0 + cw],
                in_=bass.AP(tensor=dw2img, offset=c0,
                            ap=[[4 * NCLS, 128], [NCLS, 4], [1, cw]]),
            )
            dw2_emitted[j] = True

        next_chunk = [0]

        def emit_dec2(upto, final=False):
            for t in range(next_chunk[0], upto):
                tc_ = min(128, TOKP - t * 128)
                ot = outP.tile([128, NCLS], BF16, name="ot")
                for p in range(4):
                    c0 = p * 1024
                    pw = min(1024, NCLS - c0)
                    ps = oPs.tile([128, 1024], F32, name="ops", tag="ops")
                    for half in range(2):
                        nw = min(512, pw - half * 512)
                        if nw <= 0:
                            continue
                        for kh in range(4):
                            nc.tensor.matmul(
                                ps[:tc_, half * 512: half * 512 + nw],
                                sigAll[:, kh, t * 128: t * 128 + tc_],
                                dw2sb[:, kh, c0 + half * 512: c0 + half * 512 + nw],
                                start=(kh == 0), stop=(kh == 3),
                            )
                    if p % 2 == 0:
                        nc.vector.tensor_copy(
                            out=ot[:tc_, c0: c0 + pw], in_=ps[:tc_, :pw]
                        )
                    else:
                        nc.scalar.activation(
                            ot[:tc_, c0: c0 + pw], ps[:tc_, :pw],
                            mybir.ActivationFunctionType.Copy, bias=0.0, scale=1.0,
                        )
                    if final:
                        # end-of-kernel flush: low-latency HWDGE queue,
                        # one piece per pair so copy->DMA pipelines
                        nc.sync.dma_start(
                            out=out[t * 128: t * 128 + tc_, c0: c0 + pw],
                            in_=ot[:tc_, c0: c0 + pw],
                        )
                    elif p == 1:
                        nc.gpsimd.dma_start(
                            out=out[t * 128: t * 128 + tc_, :2048],
                            in_=ot[:tc_, :2048],
                        )
                if not final:
                    nc.gpsimd.dma_start(
                        out=out[t * 128: t * 128 + tc_, 2048:],
                        in_=ot[:tc_, 2048:],
                    )
            next_chunk[0] = upto

        S = len(subgroups)
        emit_dw2(0)
        for isub in range(S):
            stage_w1(isub)
            if isub == min(2, S - 1):
                for j in range(1, 4):
                    emit_dw2(j)
            if isub >= 1:
                stage_rest(isub - 1)
            if isub == S - 1:
                # run the last subgroup's W2/dec1 BEFORE the final dec2
                # batch so its sigmoids are long done when chunk NT-1's
                # matmuls need them
                for j in range(4):
                    emit_dw2(j)
                stage_rest(isub)
                emit_dec2(NT, final=True)
            elif isub >= 2:
                e_, t0_, T_, _ = subgroups[isub - 1]
                emit_dec2((t0_ + T_) // 128)

    import bass_rust

    bass_rust.generate_event_semaphores(nc)
    return nc


_NC_CACHE = {}
_LAST_NC = None
_LAST_PERMS = None
_LAST_KS = None
_LAST_ASSIGN = None


def _get_nc(ks=None):
    global _LAST_NC
    if ks is None:
        return _LAST_NC
    ks = tuple(ks)
    if ks not in _NC_CACHE:
        _NC_CACHE[ks] = _build(ks)
    _LAST_NC = _NC_CACHE[ks]
    return _LAST_NC


def _w1_image_pair(W, nk):
    # DoubleRow pair-major: img[p, ((m*nkp+jp)*2+i)*128 + c] = W[(2jp+i)*128+p, m*128+c]
    nkp = nk // 2
    return np.ascontiguousarray(
        W.reshape(nkp, 2, 128, 4, 128).transpose(2, 3, 0, 1, 4).reshape(128, nk * 512)
    )


def _routing(inputs):
    f32 = np.float32
    x = np.asarray(inputs["fusion_hs"], f32)  # [L, B, D]
    gw = np.asarray(inputs["gate_W"], f32).astype(np.float64).reshape(L, D, NE)
    logits = np.tensordot(x.astype(np.float64), gw, axes=([0, 2], [0, 1]))
    logits += np.asarray(inputs["gate_b"], f32).astype(np.float64)
    assign = np.argmax(logits, axis=1)  # [B]
    global _LAST_ASSIGN
    _LAST_ASSIGN = assign

    ns = []
    perms = [[] for _ in range(NCORES)]
    for e in range(NE):
        idx = np.nonzero(assign == e)[0]
        if len(idx) == 0:
            ns.append(0)
            continue
        ne = -(-len(idx) // NCORES)  # ceil -> per-core count
        tot = ne * NCORES
        pad = np.full(tot, idx[0], dtype=idx.dtype)
        pad[: len(idx)] = idx
        ns.append(ne)
        for c in range(NCORES):
            perms[c].append(pad[c * ne: (c + 1) * ne])
    perms = [np.concatenate(p) for p in perms]
    return x, ns, perms


def _prep_inputs(inputs):
    global _LAST_PERMS, _LAST_KS
    f32 = np.float32
    x, ns, perms = _routing(inputs)
    _LAST_PERMS = perms
    _LAST_KS = tuple(ns)

    w1_3s = np.array(inputs["e3_W1"], f32, copy=True)
    w1_3s[: 3 * D] *= f32(np.asarray(inputs["e3_a"]).reshape(-1)[0])
    w1_3s[3 * D:] *= f32(np.asarray(inputs["e3_b"]).reshape(-1)[0])

    sc = np.float32(W1_SCALE)
    w1img = (np.concatenate(
        [
            _w1_image_pair(np.asarray(inputs["e0_W1"], f32), 12),
            _w1_image_pair(np.asarray(inputs["e1_W1"], f32), 12),
            _w1_image_pair(np.asarray(inputs["e2_W1"], f32), 24),
            _w1_image_pair(w1_3s, 24),
        ],
        axis=1,
    ) * sc).astype(NP8)
    w2img = (np.concatenate(
        [_w1_image_pair(np.asarray(inputs[f"e{e}_W2"], f32), 4) for e in range(NE)],
        axis=1,
    ) * sc).astype(NP8)
    dw1img = (_w1_image_pair(np.asarray(inputs["dec_W1"], f32), 4) * sc).astype(NP8)
    dw2img = np.ascontiguousarray(
        np.asarray(inputs["dec_W2"], f32).reshape(4, 128, NCLS)
        .transpose(1, 0, 2).reshape(128, 4 * NCLS)
    ).astype(BF)

    def cols(bs, n):
        b = np.asarray(inputs[bs], f32)
        return np.ascontiguousarray(b.reshape(n, 128).T)

    b1img = np.concatenate([cols(f"e{e}_b1", 4) for e in range(NE)], axis=1)
    b2img = np.concatenate([cols(f"e{e}_b2", 4) for e in range(NE)], axis=1)
    db1img = cols("dec_b1", 4)

    common = {
        "w1img": w1img, "w2img": w2img, "dw1img": dw1img, "dw2img": dw2img,
        "b1img": b1img, "b2img": b2img, "db1img": db1img,
    }
    xbf = x.astype(NP8)
    in_maps = []
    for c in range(NCORES):
        m = dict(common)
        xc = xbf[:, perms[c], :]                       # [6, TOKP, 512]
        m["xT"] = np.ascontiguousarray(
            xc.transpose(0, 2, 1).reshape(24 * 128, -1)
        )
        in_maps.append(m)
    return in_maps


def _host_forward(inputs, idx, assign):
    # exact fp32 forward for a few leftover tokens (device computes only
    # full 128-token chunks of dec2)
    f32 = np.float32
    x = np.asarray(inputs["fusion_hs"], f32)
    flat = np.transpose(x[:, idx, :], (1, 0, 2)).reshape(len(idx), L * D)
    out = np.empty((len(idx), NCLS), f32)
    specs = [(slice(0, 3 * D), "e0"), (slice(3 * D, 6 * D), "e1"),
             (slice(0, 6 * D), "e2"), (slice(0, 6 * D), "e3")]
    for e, (sl, _) in enumerate(specs):
        m = assign[idx] == e
        if not m.any():
            continue
        xin = flat[m][:, sl]
        W1 = np.asarray(inputs[f"e{e}_W1"], f32)
        if e == 3:
            W1 = W1.copy()
            W1[: 3 * D] *= f32(np.asarray(inputs["e3_a"]).reshape(-1)[0])
            W1[3 * D:] *= f32(np.asarray(inputs["e3_b"]).reshape(-1)[0])
        h = np.maximum(xin @ W1 + np.asarray(inputs[f"e{e}_b1"], f32), 0)
        sel = h @ np.asarray(inputs[f"e{e}_W2"], f32) + np.asarray(inputs[f"e{e}_b2"], f32)
        sig = 1.0 / (1.0 + np.exp(-(sel @ np.asarray(inputs["dec_W1"], f32)
                                    + np.asarray(inputs["dec_b1"], f32))))
        out[m] = sig @ np.asarray(inputs["dec_W2"], f32)
    return out


def kernel(**inputs):
    in_maps = _prep_inputs(inputs)
    nc = _get_nc(_LAST_KS)
    res = run_bass_kernel_spmd(nc, in_maps, core_ids=list(range(NCORES)))
    TOKP = sum(_LAST_KS)
    ndev = (TOKP // 128) * 128
    full = np.empty((B, NCLS), np.float32)
    for c in range(NCORES):
        full[_LAST_PERMS[c][:ndev]] = res.results[c]["out"][:ndev].astype(np.float32)
    if ndev < TOKP:
        tail = np.unique(np.concatenate([p[ndev:] for p in _LAST_PERMS]))
        full[tail] = _host_forward(inputs, tail, _LAST_ASSIGN)
    full += np.asarray(inputs["dec_b2"], np.float32).reshape(1, NCLS)
    return full
